# revision 9
# baseline (speedup 1.0000x reference)
"""MoE-routed DeepQNetwork kernel for 8x Trainium2 NeuronCores.

Problem: B=65536 rows, each routed to one of E=8 expert MLPs
(256 -> 64 -> 64 -> 64 -> 64 -> 64 -> 18, ReLU between layers).

Strategy (expert-grouped sharding + skewed software pipeline):
  Host: stable-sort rows by expert, pad each expert group to a multiple of
  512 rows, split the sorted+padded batch into 8 equal per-core chunks of
  nb 512-row blocks. Every block belongs to exactly ONE expert, so each
  core runs a completely static program; the per-block expert identity is
  carried purely in the per-core weight/bias input tensors.

  Device (per core, SPMD): blocks run in units of 2 (a "pair"; the last
  unit is a solo block when nb is odd). x^T arrives [256, C] fp16.
  L1 ([256->64] per block): two concurrent PE column-group matmuls (block
  a -> PSUM rows 0:64, block b -> rows 64:128), accumulated over the two
  128-row contraction chunks. L2-5 ([64->64]): h stacked [a;b] on 128
  partitions; two concurrent 64x64 tile_position matmuls — (0,0) for block
  a and (64,64) for block b — so weights are stored dense (no block-diag
  zero padding), halving weight DMA traffic. L6 ([64->18..32]): tiles
  (0,0)->PSUM 0:32 and (64,32)->PSUM 32:64.

  The program is emitted as a SKEWED PIPELINE: step s runs L6 of unit s-5,
  L5 of s-4, ..., L1 of unit s (deepest layer first). Every cross-engine
  dependency (matmul -> PSUM relu -> next matmul) then has a full step of
  slack. PSUM->SBUF bias+ReLU runs on Vector and Scalar (3 ops/unit each;
  GPSIMD cannot access PSUM). DMA rings: x halves on SP + GpSimd queues
  (grouped 1-unit-then-2-unit so the first unit lands fast), weights +
  bias on the Scalar ring with unit 0's weights alone in front (all SDMA
  engines round-robin rings at packet granularity, so the first-needed
  transfers must be small and first in their ring). fp16 outputs (rows
  0:50 only) go back on the SP ring. Dummy warm-up matmuls run during the
  initial DMA fill so the PE HAM clock-gate reaches 2.4 GHz before real
  work arrives.

  Host: unsort the fp16 outputs back to the original row order.
"""

import math
import os

import numpy as np

E = 8
D = 256
H = 64
A = 18
NCORES = 8
BLK = 512  # rows per block (matmul moving-operand free dim)

# per-unit fp16 weight tensor column layout (all on 128 partitions):
#   [0:256)   L1: block b (0=a,1=b), chunk c: col b*128+c*64 .. +64 holds
#             W1[e_b][128c:128c+128, :]  (full 128-partition lhsT)
#   [256:512) L2-5: layer li: col 256+64*li .. +64; partitions 0:64 =
#             W_{li+2}[e_a], partitions 64:128 = W_{li+2}[e_b]
#   [512:544) L6: partitions 0:64 cols 0:18 = W6[e_a] (zero padded to 32),
#             partitions 64:128 = W6[e_b]
WCOLS = 544
NWARM = 44  # PE warm-up matmuls (N=128) issued before the first real matmul
YROWS = 50  # output rows DMA'd back (0:18 = block a, 32:50 = block b)

_PROGRAM_CACHE: dict = {}
LAST_RESULTS = None  # test harness can read timing/profile info from here


def _unit_groups(npair: int):
    """transfer groups of units: first unit alone (fast pipeline start),
    then units two at a time."""
    groups = [[0]]
    p = 1
    while p < npair:
        groups.append(list(range(p, min(p + 2, npair))))
        p += 2
    return groups


def _build_program(nb: int):
    """Build the SPMD bass program for nb 512-row blocks per core."""
    import concourse.mybir as mybir
    import concourse.tile as tile
    from concourse import bacc

    f32 = mybir.dt.float32
    f16 = mybir.dt.float16
    Relu = mybir.ActivationFunctionType.Relu
    add = mybir.AluOpType.add
    amax = mybir.AluOpType.max

    npair = (nb + 1) // 2
    C = nb * BLK
    # blocks per unit (2, or 1 for a trailing solo unit)
    ublk = [2] * npair
    if nb % 2 == 1:
        ublk[-1] = 1
    ucol = [BLK * u for u in ublk]  # x columns per unit
    uoff = [0] * npair  # x column offset of unit p
    for p in range(1, npair):
        uoff[p] = uoff[p - 1] + ucol[p - 1]

    groups = _unit_groups(npair)
    grp_of = {}
    grp_start = {}
    for gi, g in enumerate(groups):
        for p in g:
            grp_of[p] = gi
            grp_start[p] = g[0]

    nc = bacc.Bacc("TRN2")
    xt0 = nc.declare_dram_parameter("xt0", [128, C], f16, isOutput=False)
    xt1 = nc.declare_dram_parameter("xt1", [128, C], f16, isOutput=False)
    wall = nc.declare_dram_parameter("wall", [128, npair * WCOLS], f16, isOutput=False)
    # per unit: cols 0:5 = b1..b5 (rows 0:64 = e_a, 64:128 = e_b), col 5 = b6
    # (rows 0:18 = b6[e_a], 32:50 = b6[e_b])
    bias = nc.declare_dram_parameter("bias", [128, npair * 6], f32, isOutput=False)
    yt = nc.declare_dram_parameter("yt", [64, npair * BLK], f16, isOutput=True)

    with tile.TileContext(nc) as tc:
        with (
            tc.tile_pool(name="wpool", bufs=1) as wpool,
            tc.tile_pool(name="xpool", bufs=1) as xpool,
            tc.tile_pool(name="hpool", bufs=10) as hpool,
            tc.tile_pool(name="opool", bufs=3) as opool,
            tc.tile_pool(name="ppool", bufs=5, space="PSUM") as ppool,
            tc.tile_pool(name="popool", bufs=1, space="PSUM") as popool,
        ):
            # ---- DMA prefetch, spread across the three DMA-capable rings
            # GpSimd: warm-up scratch memset first so warm-up can start
            scratch = wpool.tile([128, 256], f16, name="scratch", tag="scr", bufs=1)
            nc.gpsimd.memset(scratch[:, :], 0.25)

            # Scalar ring: w unit 0 first (gates the first matmul), bias,
            # then 2-unit weight groups
            bias_sb = wpool.tile([128, npair * 6], f32, name="bias_sb", tag="bias", bufs=1)
            wps = [None] * npair
            for gi, g in enumerate(groups):
                p0, p1 = g[0], g[-1] + 1
                w_g = wpool.tile(
                    [128, (p1 - p0) * WCOLS], f16, tag="wp", name=f"w_{p0}", bufs=npair
                )
                nc.scalar.dma_start(
                    out=w_g[:, :], in_=wall[:, p0 * WCOLS : p1 * WCOLS]
                )
                if gi == 0:
                    nc.scalar.dma_start(out=bias_sb[:, :], in_=bias[:, :])
                for p in g:
                    wps[p] = w_g[:, (p - p0) * WCOLS : (p - p0 + 1) * WCOLS]

            # SP ring: xc0 groups;  GpSimd ring: xc1 groups
            xg0, xg1 = [], []
            for gi, g in enumerate(groups):
                p0 = g[0]
                cols = sum(ucol[p] for p in g)
                xc0 = xpool.tile([128, cols], f16, tag=f"xc0g{gi}", name=f"xc0_{p0}", bufs=1)
                xc1 = xpool.tile([128, cols], f16, tag=f"xc1g{gi}", name=f"xc1_{p0}", bufs=1)
                nc.sync.dma_start(out=xc0[:, :], in_=xt0[:, uoff[p0] : uoff[p0] + cols])
                nc.gpsimd.dma_start(out=xc1[:, :], in_=xt1[:, uoff[p0] : uoff[p0] + cols])
                xg0.append(xc0)
                xg1.append(xc1)

            # ---- PE warm-up: garbage matmuls to lift the HAM clock gate
            # while the first x/w DMAs are in flight.
            warm_ps = popool.tile([64, BLK], f32, name="warm_ps", tag="warm", bufs=1)
            for i in range(NWARM):
                nc.tensor.matmul(
                    out=warm_ps[0:64, 0:128],
                    lhsT=scratch[:, 0:64],
                    rhs=scratch[:, 128:256],
                    start=True,
                    stop=True,
                )

            # ---- Skewed pipeline: step s emits L6_{s-5} ... L1_s
            # (GPSIMD cannot access PSUM, so only DVE + ACT do the relus;
            # (p + l//2) % 2 gives each engine 3 ops per unit AND per step)
            relu_eng = [nc.vector, nc.scalar]
            bof = [6 * p for p in range(npair)]
            hcur = [None] * npair

            def do_relu(p, l, h, ph, hp=128):
                """bias + relu (or plain bias add for l==6), engine rotated"""
                eng = relu_eng[(p + l // 2) % 2]
                if l == 6:
                    bap = bias_sb[0:64, bof[p] + 5 : bof[p] + 6]
                    if eng is nc.scalar:
                        nc.scalar.add(h[:, :], ph[:, :], bap)
                    else:
                        eng.tensor_scalar(h[:, :], ph[:, :], bap, None, op0=add)
                else:
                    bap = bias_sb[0:hp, bof[p] + l - 1 : bof[p] + l]
                    if eng is nc.scalar:
                        nc.scalar.activation(h[:, :], ph[:, :], Relu, bias=bap)
                    else:
                        eng.tensor_scalar(h[:, :], ph[:, :], bap, 0.0, op0=add, op1=amax)

            def emit_layer(l, p):
                solo = ublk[p] == 1
                hp = 64 if solo else 128
                if l == 1:
                    gi = grp_of[p]
                    off = uoff[p] - uoff[grp_start[p]]
                    ph1 = ppool.tile([128, BLK], f32, tag="ph", name=f"ph1_{p}")
                    for c, xt in ((0, xg0[gi]), (1, xg1[gi])):
                        for blk in range(ublk[p]):
                            nc.tensor.matmul(
                                out=ph1[blk * 64 : (blk + 1) * 64, :],
                                lhsT=wps[p][:, blk * 128 + c * 64 : blk * 128 + (c + 1) * 64],
                                rhs=xt[:, off + blk * BLK : off + (blk + 1) * BLK],
                                start=(c == 0),
                                stop=(c == 1),
                            )
                    h1 = hpool.tile([128, BLK], f16, tag="h", name=f"h1_{p}")
                    do_relu(p, 1, h1[0:hp, :], ph1[0:hp, :], hp)
                    hcur[p] = h1
                elif l <= 5:
                    li = l - 2
                    wc = 256 + li * 64
                    ph = ppool.tile([128, BLK], f32, tag="ph", name=f"ph{l}_{p}")
                    nc.tensor.matmul(
                        out=ph[0:64, :],
                        lhsT=wps[p][0:64, wc : wc + 64],
                        rhs=hcur[p][0:64, :],
                        start=True,
                        stop=True,
                    )
                    if not solo:
                        nc.tensor.matmul(
                            out=ph[64:128, :],
                            lhsT=wps[p][64:128, wc : wc + 64],
                            rhs=hcur[p][64:128, :],
                            start=True,
                            stop=True,
                        )
                    h = hpool.tile([128, BLK], f16, tag="h", name=f"h{l}_{p}")
                    do_relu(p, l, h[0:hp, :], ph[0:hp, :], hp)
                    hcur[p] = h
                else:  # l == 6
                    po = popool.tile([64, BLK], f32, tag="po", name=f"po_{p}", bufs=2)
                    nc.tensor.matmul(
                        out=po[0:32, :],
                        lhsT=wps[p][0:64, 512:544],
                        rhs=hcur[p][0:64, :],
                        start=True,
                        stop=True,
                    )
                    if not solo:
                        nc.tensor.matmul(
                            out=po[32:64, :],
                            lhsT=wps[p][64:128, 512:544],
                            rhs=hcur[p][64:128, :],
                            start=True,
                            stop=True,
                        )
                    o_p = opool.tile([64, BLK], f16, tag="o", name=f"o_{p}")
                    do_relu(p, 6, o_p[:, :], po[:, :])
                    nc.sync.dma_start(
                        out=yt[0:YROWS, p * BLK : (p + 1) * BLK], in_=o_p[0:YROWS, :]
                    )

            STAGES = 6
            for s in range(npair + STAGES - 1):
                for l in range(STAGES, 0, -1):
                    p = s - (l - 1)
                    if 0 <= p < npair:
                        emit_layer(l, p)

    nc.compile()
    return nc


def _get_program(nb: int):
    if nb not in _PROGRAM_CACHE:
        _PROGRAM_CACHE[nb] = _build_program(nb)
    return _PROGRAM_CACHE[nb]


def _prepare(state, rm_state, W1, b1, W2, b2, W3, b3, W4, b4, W5, b5, W6, b6):
    state = np.ascontiguousarray(np.asarray(state, dtype=np.float32))
    rm = np.asarray(rm_state).reshape(-1).astype(np.int64)
    Ws = [np.asarray(w, dtype=np.float32) for w in (W1, W2, W3, W4, W5, W6)]
    bs = [np.asarray(b, dtype=np.float32) for b in (b1, b2, b3, b4, b5, b6)]
    B = state.shape[0]
    X = state.reshape(B, D)

    # ---- host-side routing: stable sort rows by expert, pad groups to BLK
    order = np.argsort(rm, kind="stable")
    counts = np.bincount(rm, minlength=E)
    caps = ((counts + BLK - 1) // BLK) * BLK
    caps = np.maximum(caps, BLK)  # empty groups still occupy one (zero) block
    T0 = int(caps.sum())
    C = math.ceil(T0 / NCORES / BLK) * BLK
    T = NCORES * C
    caps[E - 1] += T - T0  # extend last group's padding to fill all cores
    base = np.zeros(E, dtype=np.int64)
    base[1:] = np.cumsum(caps)[:-1]
    csum = np.zeros(E, dtype=np.int64)
    csum[1:] = np.cumsum(counts)[:-1]
    sorted_expert = rm[order]
    pos_sorted = base[sorted_expert] + (np.arange(B) - csum[sorted_expert])

    Xp = np.zeros((T, D), np.float16)
    Xp[pos_sorted] = X[order].astype(np.float16)
    blk_expert = np.zeros(T // BLK, np.int64)
    for e in range(E):
        blk_expert[base[e] // BLK : (base[e] + caps[e]) // BLK] = e

    W16 = [w.astype(np.float16) for w in Ws]

    nb = C // BLK
    npair = (nb + 1) // 2

    in_maps = []
    for core in range(NCORES):
        xt = np.ascontiguousarray(Xp[core * C : (core + 1) * C].T)  # [D, C] fp16
        be = blk_expert[core * nb : (core + 1) * nb]

        wh = np.zeros((128, npair * WCOLS), np.float16)
        bh = np.zeros((128, npair * 6), np.float32)
        for p in range(npair):
            w = wh[:, p * WCOLS : (p + 1) * WCOLS]
            bb = bh[:, p * 6 : (p + 1) * 6]
            ea = be[2 * p]
            has_b = 2 * p + 1 < nb
            eb = be[2 * p + 1] if has_b else ea
            blocks = ((0, ea), (1, eb)) if has_b else ((0, ea),)
            for blk, e in blocks:
                for c in range(2):
                    w[:, blk * 128 + c * 64 : blk * 128 + (c + 1) * 64] = W16[0][
                        e, 128 * c : 128 * (c + 1), :
                    ]
            for li in range(4):
                wc = 256 + li * 64
                w[0:64, wc : wc + 64] = W16[li + 1][ea]
                if has_b:
                    w[64:128, wc : wc + 64] = W16[li + 1][eb]
            w[0:64, 512 : 512 + A] = W16[5][ea]
            if has_b:
                w[64:128, 512 : 512 + A] = W16[5][eb]
            for li in range(5):
                bb[0:64, li] = bs[li][ea]
                bb[64:128, li] = bs[li][eb]
            bb[0:A, 5] = bs[5][ea]
            bb[32 : 32 + A, 5] = bs[5][eb]

        in_maps.append(
            {
                "xt0": np.ascontiguousarray(xt[0:128]),
                "xt1": np.ascontiguousarray(xt[128:256]),
                "wall": wh,
                "bias": bh,
            }
        )

    meta = dict(B=B, C=C, T=T, nb=nb, npair=npair, order=order, pos_sorted=pos_sorted)
    return in_maps, meta


def _finalize(results, meta):
    """results: list (per core) of dicts with 'yt' [64, npair*BLK] f16 arrays."""
    B, C, T, nb, npair = (meta[k] for k in ("B", "C", "T", "nb", "npair"))
    Yp = np.zeros((T, A), np.float32)
    for core in range(NCORES):
        ytc = np.asarray(results[core]["yt"], dtype=np.float32)
        for p in range(npair):
            cols = slice(p * BLK, (p + 1) * BLK)
            dst = core * C + 2 * p * BLK
            Yp[dst : dst + BLK] = ytc[0:A, cols].T
            if 2 * p + 1 < nb:
                Yp[dst + BLK : dst + 2 * BLK] = ytc[32 : 32 + A, cols].T

    y = np.zeros((B, A), np.float32)
    y[meta["order"]] = Yp[meta["pos_sorted"]]
    return y


def kernel(state, rm_state, W1, b1, W2, b2, W3, b3, W4, b4, W5, b5, W6, b6):
    global LAST_RESULTS
    from concourse.bass_utils import run_bass_kernel_spmd

    in_maps, meta = _prepare(
        state, rm_state, W1, b1, W2, b2, W3, b3, W4, b4, W5, b5, W6, b6
    )
    nc = _get_program(meta["nb"])
    trace = bool(os.environ.get("KERNEL_TRACE"))
    res = run_bass_kernel_spmd(nc, in_maps, core_ids=list(range(NCORES)), trace=trace)
    LAST_RESULTS = res
    return _finalize(res.results, meta)


# revision 11
# speedup vs baseline: 1.0484x; 1.0484x over previous
"""MoE-routed DeepQNetwork kernel for 8x Trainium2 NeuronCores.

Problem: B=65536 rows, each routed to one of E=8 expert MLPs
(256 -> 64 -> 64 -> 64 -> 64 -> 64 -> 18, ReLU between layers).

Strategy (expert-grouped sharding + skewed software pipeline):
  Host: stable-sort rows by expert, pad each expert group to a multiple of
  512 rows, split the sorted+padded batch into 8 equal per-core chunks of
  nb 512-row blocks. Every block belongs to exactly ONE expert, so each
  core runs a completely static program; the per-block expert identity is
  carried purely in the per-core weight/bias input tensors.

  Device (per core, SPMD): blocks run in units of 2 (a "pair"; the last
  unit is a solo block when nb is odd). x^T arrives [256, C] fp16.
  L1 ([256->64] per block): two concurrent PE column-group matmuls (block
  a -> PSUM rows 0:64, block b -> rows 64:128), accumulated over the two
  128-row contraction chunks. L2-5 ([64->64]): h stacked [a;b] on 128
  partitions; two concurrent 64x64 tile_position matmuls — (0,0) for block
  a and (64,64) for block b — so weights are stored dense (no block-diag
  zero padding), halving weight DMA traffic. L6 ([64->18..32]): tiles
  (0,0)->PSUM 0:32 and (64,32)->PSUM 32:64.

  The program is emitted as a SKEWED PIPELINE: step s runs L6 of unit s-5,
  L5 of s-4, ..., L1 of unit s (deepest layer first). Every cross-engine
  dependency (matmul -> PSUM relu -> next matmul) then has a full step of
  slack. PSUM->SBUF bias+ReLU runs on Vector and Scalar (3 ops/unit each;
  GPSIMD cannot access PSUM). DMA rings: x halves on SP + GpSimd queues
  (grouped 1-unit-then-2-unit so the first unit lands fast), weights +
  bias on the Scalar ring with unit 0's weights alone in front (all SDMA
  engines round-robin rings at packet granularity, so the first-needed
  transfers must be small and first in their ring). fp16 outputs (rows
  0:50 only) go back on the SP ring. Dummy warm-up matmuls run during the
  initial DMA fill so the PE HAM clock-gate reaches 2.4 GHz before real
  work arrives.

  Host: unsort the fp16 outputs back to the original row order.
"""

import math
import os

import numpy as np

E = 8
D = 256
H = 64
A = 18
NCORES = 8
BLK = 512  # rows per block (matmul moving-operand free dim)

# per-unit fp16 weight tensor column layout (all on 128 partitions):
#   [0:256)   L1: block b (0=a,1=b), chunk c: col b*128+c*64 .. +64 holds
#             W1[e_b][128c:128c+128, :]  (full 128-partition lhsT)
#   [256:512) L2-5: layer li: col 256+64*li .. +64; partitions 0:64 =
#             W_{li+2}[e_a], partitions 64:128 = W_{li+2}[e_b]
#   [512:544) L6: partitions 0:64 cols 0:18 = W6[e_a] (zero padded to 32),
#             partitions 64:128 = W6[e_b]
WCOLS = 544
NWARM = 28  # PE warm-up matmuls (N=128) issued before the first real matmul
YROWS = 50  # output rows DMA'd back (0:18 = block a, 32:50 = block b)

_PROGRAM_CACHE: dict = {}
LAST_RESULTS = None  # test harness can read timing/profile info from here


def _unit_groups(npair: int):
    """transfer groups of units: first unit alone (fast pipeline start),
    then units two at a time."""
    groups = [[0]]
    p = 1
    while p < npair:
        groups.append(list(range(p, min(p + 2, npair))))
        p += 2
    return groups


def _build_program(nb: int):
    """Build the SPMD bass program for nb 512-row blocks per core."""
    import concourse.mybir as mybir
    import concourse.tile as tile
    from concourse import bacc

    f32 = mybir.dt.float32
    f16 = mybir.dt.float16
    Relu = mybir.ActivationFunctionType.Relu
    add = mybir.AluOpType.add
    amax = mybir.AluOpType.max

    npair = (nb + 1) // 2
    C = nb * BLK
    # blocks per unit (2, or 1 for a trailing solo unit)
    ublk = [2] * npair
    if nb % 2 == 1:
        ublk[-1] = 1
    ucol = [BLK * u for u in ublk]  # x columns per unit
    uoff = [0] * npair  # x column offset of unit p
    for p in range(1, npair):
        uoff[p] = uoff[p - 1] + ucol[p - 1]

    groups = _unit_groups(npair)
    grp_of = {}
    grp_start = {}
    for gi, g in enumerate(groups):
        for p in g:
            grp_of[p] = gi
            grp_start[p] = g[0]

    nc = bacc.Bacc("TRN2")
    xt0 = nc.declare_dram_parameter("xt0", [128, C], f16, isOutput=False)
    xt1 = nc.declare_dram_parameter("xt1", [128, C], f16, isOutput=False)
    wall = nc.declare_dram_parameter("wall", [128, npair * WCOLS], f16, isOutput=False)
    # per unit: cols 0:5 = b1..b5 (rows 0:64 = e_a, 64:128 = e_b), col 5 = b6
    # (rows 0:18 = b6[e_a], 32:50 = b6[e_b])
    bias = nc.declare_dram_parameter("bias", [128, npair * 6], f32, isOutput=False)
    yt = nc.declare_dram_parameter("yt", [64, npair * BLK], f16, isOutput=True)

    with tile.TileContext(nc) as tc:
        with (
            tc.tile_pool(name="wpool", bufs=1) as wpool,
            tc.tile_pool(name="xpool", bufs=1) as xpool,
            tc.tile_pool(name="hpool", bufs=10) as hpool,
            tc.tile_pool(name="opool", bufs=3) as opool,
            tc.tile_pool(name="ppool", bufs=5, space="PSUM") as ppool,
            tc.tile_pool(name="popool", bufs=1, space="PSUM") as popool,
        ):
            # ---- DMA prefetch. Only TWO rings (SP + GpSimd) carry input:
            # all SDMA engines round-robin the active rings at packet
            # granularity, so a third ring with small packets would starve.
            # Each ring is FIFO, so transfers are enqueued in need-order,
            # with weight groups split alternately between the rings.
            # GpSimd: warm-up scratch memset first so warm-up can start.
            scratch = wpool.tile([128, 256], f16, name="scratch", tag="scr", bufs=1)
            nc.gpsimd.memset(scratch[:, :], 0.25)

            bias_sb = wpool.tile([128, npair * 6], f32, name="bias_sb", tag="bias", bufs=1)
            nc.sync.dma_start(out=bias_sb[:, :], in_=bias[:, :])

            wps = [None] * npair
            wtiles = []
            for gi, g in enumerate(groups):
                p0, p1 = g[0], g[-1] + 1
                w_g = wpool.tile(
                    [128, (p1 - p0) * WCOLS], f16, tag="wp", name=f"w_{p0}", bufs=npair
                )
                wtiles.append(w_g)
                for p in g:
                    wps[p] = w_g[:, (p - p0) * WCOLS : (p - p0 + 1) * WCOLS]

            xg0, xg1 = [], []
            for gi, g in enumerate(groups):
                p0 = g[0]
                cols = sum(ucol[p] for p in g)
                xc0 = xpool.tile([128, cols], f16, tag=f"xc0g{gi}", name=f"xc0_{p0}", bufs=1)
                xc1 = xpool.tile([128, cols], f16, tag=f"xc1g{gi}", name=f"xc1_{p0}", bufs=1)
                xg0.append(xc0)
                xg1.append(xc1)

            # need-ordered interleave: w group gi rides gpsimd for even gi
            # (w_g0 ahead of xc1_g0 gates the very first matmul), sync for
            # odd gi, always enqueued just before the x group it feeds.
            for gi, g in enumerate(groups):
                p0 = g[0]
                cols = sum(ucol[p] for p in g)
                weng = nc.gpsimd if gi % 2 == 0 else nc.sync
                weng.dma_start(
                    out=wtiles[gi][:, :],
                    in_=wall[:, g[0] * WCOLS : (g[-1] + 1) * WCOLS],
                )
                nc.sync.dma_start(out=xg0[gi][:, :], in_=xt0[:, uoff[p0] : uoff[p0] + cols])
                nc.gpsimd.dma_start(out=xg1[gi][:, :], in_=xt1[:, uoff[p0] : uoff[p0] + cols])

            # ---- PE warm-up: garbage matmuls to lift the HAM clock gate
            # while the first x/w DMAs are in flight.
            warm_ps = popool.tile([64, BLK], f32, name="warm_ps", tag="warm", bufs=1)
            for i in range(NWARM):
                nc.tensor.matmul(
                    out=warm_ps[0:64, 0:128],
                    lhsT=scratch[:, 0:64],
                    rhs=scratch[:, 128:256],
                    start=True,
                    stop=True,
                )

            # ---- Skewed pipeline: step s emits L6_{s-5} ... L1_s
            # (GPSIMD cannot access PSUM, so only DVE + ACT do the relus;
            # (p + l//2) % 2 gives each engine 3 ops per unit AND per step)
            relu_eng = [nc.vector, nc.scalar]
            bof = [6 * p for p in range(npair)]
            hcur = [None] * npair

            def do_relu(p, l, h, ph, hp=128):
                """bias + relu (or plain bias add for l==6), engine rotated"""
                eng = relu_eng[(p + l // 2) % 2]
                if l == 6:
                    bap = bias_sb[0:64, bof[p] + 5 : bof[p] + 6]
                    if eng is nc.scalar:
                        nc.scalar.add(h[:, :], ph[:, :], bap)
                    else:
                        eng.tensor_scalar(h[:, :], ph[:, :], bap, None, op0=add)
                else:
                    bap = bias_sb[0:hp, bof[p] + l - 1 : bof[p] + l]
                    if eng is nc.scalar:
                        nc.scalar.activation(h[:, :], ph[:, :], Relu, bias=bap)
                    else:
                        eng.tensor_scalar(h[:, :], ph[:, :], bap, 0.0, op0=add, op1=amax)

            def emit_layer(l, p):
                solo = ublk[p] == 1
                hp = 64 if solo else 128
                if l == 1:
                    gi = grp_of[p]
                    off = uoff[p] - uoff[grp_start[p]]
                    ph1 = ppool.tile([128, BLK], f32, tag="ph", name=f"ph1_{p}")
                    for c, xt in ((0, xg0[gi]), (1, xg1[gi])):
                        for blk in range(ublk[p]):
                            nc.tensor.matmul(
                                out=ph1[blk * 64 : (blk + 1) * 64, :],
                                lhsT=wps[p][:, blk * 128 + c * 64 : blk * 128 + (c + 1) * 64],
                                rhs=xt[:, off + blk * BLK : off + (blk + 1) * BLK],
                                start=(c == 0),
                                stop=(c == 1),
                            )
                    h1 = hpool.tile([128, BLK], f16, tag="h", name=f"h1_{p}")
                    do_relu(p, 1, h1[0:hp, :], ph1[0:hp, :], hp)
                    hcur[p] = h1
                elif l <= 5:
                    li = l - 2
                    wc = 256 + li * 64
                    ph = ppool.tile([128, BLK], f32, tag="ph", name=f"ph{l}_{p}")
                    nc.tensor.matmul(
                        out=ph[0:64, :],
                        lhsT=wps[p][0:64, wc : wc + 64],
                        rhs=hcur[p][0:64, :],
                        start=True,
                        stop=True,
                    )
                    if not solo:
                        nc.tensor.matmul(
                            out=ph[64:128, :],
                            lhsT=wps[p][64:128, wc : wc + 64],
                            rhs=hcur[p][64:128, :],
                            start=True,
                            stop=True,
                        )
                    h = hpool.tile([128, BLK], f16, tag="h", name=f"h{l}_{p}")
                    do_relu(p, l, h[0:hp, :], ph[0:hp, :], hp)
                    hcur[p] = h
                else:  # l == 6
                    po = popool.tile([64, BLK], f32, tag="po", name=f"po_{p}", bufs=2)
                    nc.tensor.matmul(
                        out=po[0:32, :],
                        lhsT=wps[p][0:64, 512:544],
                        rhs=hcur[p][0:64, :],
                        start=True,
                        stop=True,
                    )
                    if not solo:
                        nc.tensor.matmul(
                            out=po[32:64, :],
                            lhsT=wps[p][64:128, 512:544],
                            rhs=hcur[p][64:128, :],
                            start=True,
                            stop=True,
                        )
                    o_p = opool.tile([64, BLK], f16, tag="o", name=f"o_{p}")
                    do_relu(p, 6, o_p[:, :], po[:, :])
                    nc.sync.dma_start(
                        out=yt[0:YROWS, p * BLK : (p + 1) * BLK], in_=o_p[0:YROWS, :]
                    )

            STAGES = 6
            for s in range(npair + STAGES - 1):
                for l in range(STAGES, 0, -1):
                    p = s - (l - 1)
                    if 0 <= p < npair:
                        emit_layer(l, p)

    nc.compile()
    return nc


def _get_program(nb: int):
    if nb not in _PROGRAM_CACHE:
        _PROGRAM_CACHE[nb] = _build_program(nb)
    return _PROGRAM_CACHE[nb]


def _prepare(state, rm_state, W1, b1, W2, b2, W3, b3, W4, b4, W5, b5, W6, b6):
    state = np.ascontiguousarray(np.asarray(state, dtype=np.float32))
    rm = np.asarray(rm_state).reshape(-1).astype(np.int64)
    Ws = [np.asarray(w, dtype=np.float32) for w in (W1, W2, W3, W4, W5, W6)]
    bs = [np.asarray(b, dtype=np.float32) for b in (b1, b2, b3, b4, b5, b6)]
    B = state.shape[0]
    X = state.reshape(B, D)

    # ---- host-side routing: stable sort rows by expert, pad groups to BLK
    order = np.argsort(rm, kind="stable")
    counts = np.bincount(rm, minlength=E)
    caps = ((counts + BLK - 1) // BLK) * BLK
    caps = np.maximum(caps, BLK)  # empty groups still occupy one (zero) block
    T0 = int(caps.sum())
    C = math.ceil(T0 / NCORES / BLK) * BLK
    T = NCORES * C
    caps[E - 1] += T - T0  # extend last group's padding to fill all cores
    base = np.zeros(E, dtype=np.int64)
    base[1:] = np.cumsum(caps)[:-1]
    csum = np.zeros(E, dtype=np.int64)
    csum[1:] = np.cumsum(counts)[:-1]
    sorted_expert = rm[order]
    pos_sorted = base[sorted_expert] + (np.arange(B) - csum[sorted_expert])

    Xp = np.zeros((T, D), np.float16)
    Xp[pos_sorted] = X[order].astype(np.float16)
    blk_expert = np.zeros(T // BLK, np.int64)
    for e in range(E):
        blk_expert[base[e] // BLK : (base[e] + caps[e]) // BLK] = e

    W16 = [w.astype(np.float16) for w in Ws]

    nb = C // BLK
    npair = (nb + 1) // 2

    in_maps = []
    for core in range(NCORES):
        xt = np.ascontiguousarray(Xp[core * C : (core + 1) * C].T)  # [D, C] fp16
        be = blk_expert[core * nb : (core + 1) * nb]

        wh = np.zeros((128, npair * WCOLS), np.float16)
        bh = np.zeros((128, npair * 6), np.float32)
        for p in range(npair):
            w = wh[:, p * WCOLS : (p + 1) * WCOLS]
            bb = bh[:, p * 6 : (p + 1) * 6]
            ea = be[2 * p]
            has_b = 2 * p + 1 < nb
            eb = be[2 * p + 1] if has_b else ea
            blocks = ((0, ea), (1, eb)) if has_b else ((0, ea),)
            for blk, e in blocks:
                for c in range(2):
                    w[:, blk * 128 + c * 64 : blk * 128 + (c + 1) * 64] = W16[0][
                        e, 128 * c : 128 * (c + 1), :
                    ]
            for li in range(4):
                wc = 256 + li * 64
                w[0:64, wc : wc + 64] = W16[li + 1][ea]
                if has_b:
                    w[64:128, wc : wc + 64] = W16[li + 1][eb]
            w[0:64, 512 : 512 + A] = W16[5][ea]
            if has_b:
                w[64:128, 512 : 512 + A] = W16[5][eb]
            for li in range(5):
                bb[0:64, li] = bs[li][ea]
                bb[64:128, li] = bs[li][eb]
            bb[0:A, 5] = bs[5][ea]
            bb[32 : 32 + A, 5] = bs[5][eb]

        in_maps.append(
            {
                "xt0": np.ascontiguousarray(xt[0:128]),
                "xt1": np.ascontiguousarray(xt[128:256]),
                "wall": wh,
                "bias": bh,
            }
        )

    meta = dict(B=B, C=C, T=T, nb=nb, npair=npair, order=order, pos_sorted=pos_sorted)
    return in_maps, meta


def _finalize(results, meta):
    """results: list (per core) of dicts with 'yt' [64, npair*BLK] f16 arrays."""
    B, C, T, nb, npair = (meta[k] for k in ("B", "C", "T", "nb", "npair"))
    Yp = np.zeros((T, A), np.float32)
    for core in range(NCORES):
        ytc = np.asarray(results[core]["yt"], dtype=np.float32)
        for p in range(npair):
            cols = slice(p * BLK, (p + 1) * BLK)
            dst = core * C + 2 * p * BLK
            Yp[dst : dst + BLK] = ytc[0:A, cols].T
            if 2 * p + 1 < nb:
                Yp[dst + BLK : dst + 2 * BLK] = ytc[32 : 32 + A, cols].T

    y = np.zeros((B, A), np.float32)
    y[meta["order"]] = Yp[meta["pos_sorted"]]
    return y


def kernel(state, rm_state, W1, b1, W2, b2, W3, b3, W4, b4, W5, b5, W6, b6):
    global LAST_RESULTS
    from concourse.bass_utils import run_bass_kernel_spmd

    in_maps, meta = _prepare(
        state, rm_state, W1, b1, W2, b2, W3, b3, W4, b4, W5, b5, W6, b6
    )
    nc = _get_program(meta["nb"])
    trace = bool(os.environ.get("KERNEL_TRACE"))
    res = run_bass_kernel_spmd(nc, in_maps, core_ids=list(range(NCORES)), trace=trace)
    LAST_RESULTS = res
    return _finalize(res.results, meta)
